# revision 18
# baseline (speedup 1.0000x reference)
"""Trainium2 Bass kernel for nn_DotProductAttention_57853209477398.

Reference computation (per batch b):
    kprojT = W1 @ keys[b].T                      # [128, 1024]   (d, f)
    scoresT[k, q] = sum_d kprojT[d,k] * Q[d,q]   # [1024, 1024]  (Fk, Fq) / sqrt(128)
    attnT = softmax over q (free dim)            # softmax axis=1 of scores == free dim of scoresT
    out[q, l] = sum_k attnT[k,q] * V[l,k]        # [1024, 512]

Sharding: data-parallel over batch B=32 across 8 cores (4 batches/core).
All matmuls run as float32r (~13-bit-mantissa rounded fp32) at full PE rate.
"""

import numpy as np

import concourse.bacc as bacc
import concourse.mybir as mybir
from concourse.tile import TileContext
from concourse.masks import make_identity

B, LQ, FQ = 32, 128, 1024
LK, FK = 512, 1024
LV, FV = 512, 1024
DP = 128

NCORES = 8
NB = B // NCORES  # batches per core
P = 128
N = 512  # max fp32 moving free dim per matmul
F32 = mybir.dt.float32
F32R = mybir.dt.float32r
INV_SQRT_D = 1.0 / np.sqrt(np.float32(DP))

KC = LK // P  # 4: l-chunks of keys/values rows
KF = FK // P  # 8: k-chunks (scoresT partition tiles)
QC = FQ // P  # 8: q-chunks of the output


def build_kernel(reps=1, dma_only=False, compute_only=False):
    nc = bacc.Bacc("TRN2", target_bir_lowering=False)
    q_d = nc.dram_tensor("queries", [NB, LQ, FQ], F32, kind="ExternalInput")
    k_d = nc.dram_tensor("keys", [NB, LK, FK], F32, kind="ExternalInput")
    v_d = nc.dram_tensor("values", [NB, LV, FV], F32, kind="ExternalInput")
    w_d = nc.dram_tensor("w1t", [LK, DP], F32, kind="ExternalInput")
    o_d = nc.dram_tensor("out", [NB, FQ, LV], F32, kind="ExternalOutput")

    with TileContext(nc) as tc:
        with (
            tc.tile_pool(name="const", bufs=1) as cpool,
            tc.tile_pool(name="io", bufs=2) as io,
            tc.tile_pool(name="work", bufs=2) as work,
            tc.tile_pool(name="ps_big", bufs=2, space="PSUM") as ps_big,
            tc.tile_pool(name="ps_vt", bufs=2, space="PSUM") as ps_vt,
            tc.tile_pool(name="ps_o", bufs=2, space="PSUM") as ps_o,
        ):
            ident = cpool.tile([P, P], F32)
            make_identity(nc, ident[:])
            w1t_raw = cpool.tile([P, KC, DP], F32)
            nc.sync.dma_start(w1t_raw[:], w_d.ap().rearrange("(c p) d -> p c d", p=P))
            w1t = cpool.tile([P, KC, DP], F32R)
            nc.vector.tensor_copy(w1t[:], w1t_raw[:])  # round to f32r
            o_static = cpool.tile([P, LV], F32)
            nc.gpsimd.memset(o_static[:], 0.5)

            import contextlib

            loop_cm = (
                tc.For_i(0, reps, 1, name="rep")
                if reps > 1
                else contextlib.nullcontext()
            )
            with loop_cm:
                for b in range(NB):
                    # fp32 DMA lands in staging tiles; a DVE copy rounds into
                    # separate f32r tiles (FP32r matmul inputs must be produced
                    # by a rounding compute op, not DMA). K chunks are loaded
                    # first: kprojT is the first consumer.
                    kr = work.tile([P, KC, FK], F32R, tag="kr")
                    k_src = k_d.ap()[b].rearrange("(c p) f -> p c f", p=P)
                    for lc in range(KC):
                        st = io.tile([P, FQ], F32, tag="stage", bufs=4)
                        if compute_only:
                            nc.gpsimd.memset(st[:], 0.25)
                        else:
                            nc.sync.dma_start(st[:], k_src[:, lc])
                        if not dma_only:
                            nc.vector.tensor_copy(kr[:, lc], st[:])

                    qr = work.tile([P, FQ], F32R, tag="qr", bufs=1)
                    st = io.tile([P, FQ], F32, tag="stage", bufs=4)
                    if compute_only:
                        nc.gpsimd.memset(st[:], 0.25)
                    else:
                        nc.sync.dma_start(st[:], q_d.ap()[b])
                    if not dma_only:
                        nc.vector.tensor_copy(qr[:], st[:])

                    v = io.tile([P, KC, FV], F32, tag="v")
                    v_src = v_d.ap()[b].rearrange("(c p) f -> p c f", p=P)
                    for lc in range(KC):
                        if compute_only:
                            nc.gpsimd.memset(v[:, lc], 0.25)
                        else:
                            nc.sync.dma_start(v[:, lc], v_src[:, lc])

                    if dma_only:
                        for qc in range(QC):
                            nc.sync.dma_start(
                                o_d.ap()[b, qc * P : (qc + 1) * P], o_static[:]
                            )
                        continue
                    # ---- kprojT[d, f] = sum_l W1T[l, d] * K[l, f] ----
                    kp_ps = ps_big.tile([P, FK], F32, tag="big")
                    for lc in range(KC):  # 4 (chunk-major: start on first K chunk)
                        for fc in range(FK // N):  # 2
                            nc.tensor.matmul(
                                kp_ps[:, fc * N : (fc + 1) * N],
                                w1t[:, lc],
                                kr[:, lc, fc * N : (fc + 1) * N],
                                start=(lc == 0),
                                stop=(lc == KC - 1),
                            )
                    kproj = work.tile([P, FK], F32R, tag="kproj", bufs=1)
                    # PSUM->SBUF + round to f32r; split across DVE/ACT to
                    # halve the critical-path latency into the scores matmuls
                    nc.vector.tensor_copy(kproj[:, :N], kp_ps[:, :N])
                    nc.scalar.copy(kproj[:, N:], kp_ps[:, N:])

                    # ---- scoresT -> exp/row-sum -> per-kc recip -> scaled V^T ----
                    attn = work.tile([P, KF, FQ], F32R, tag="attn")
                    den = work.tile([P, KF], F32, tag="den")
                    rden = work.tile([P, KF], F32, tag="rden")
                    vt = work.tile([P, KF, LV], F32R, tag="vt")
                    o_half = [
                        ps_o.tile([P, LV], F32, tag="o_ps", name=f"o_half{qg}")
                        for qg in range(2)
                    ]
                    for kc in range(KF):  # 8
                        sc_ps = ps_big.tile([P, FQ], F32, tag="big")
                        for qc in range(FQ // N):  # 2
                            nc.tensor.matmul(
                                sc_ps[:, qc * N : (qc + 1) * N],
                                kproj[:, kc * P : (kc + 1) * P],
                                qr[:, qc * N : (qc + 1) * N],
                                start=True,
                                stop=True,
                            )
                        nc.scalar.activation(
                            attn[:, kc],
                            sc_ps[:],
                            mybir.ActivationFunctionType.Exp,
                            scale=float(INV_SQRT_D),
                            accum_out=den[:, kc : kc + 1],
                        )
                        nc.vector.reciprocal(rden[:, kc : kc + 1], den[:, kc : kc + 1])
                        vt_ps = ps_vt.tile([P, LV], F32, tag="vt_ps")
                        for lc in range(KC):
                            nc.tensor.transpose(
                                vt_ps[:, lc * P : (lc + 1) * P],
                                v[:, lc, kc * P : (kc + 1) * P],
                                ident[:],
                            )
                        nc.vector.tensor_scalar_mul(
                            vt[:, kc], vt_ps[:], rden[:, kc : kc + 1]
                        )
                        # first-half out matmuls ride along per kc: fills PE
                        # during the ACT-paced exp chain and shortens the
                        # batch tail to the remaining half
                        for qg in range(2):
                            nc.tensor.matmul(
                                o_half[qg],
                                attn[:, kc, qg * P : (qg + 1) * P],
                                vt[:, kc],
                                start=(kc == 0),
                                stop=(kc == KF - 1),
                            )

                    # ---- out[q, l] = sum_k attnT[k, q] * VT[k, l] ----
                    for qc in range(QC):  # 8
                        if qc < 2:
                            o_ps = o_half[qc]
                        else:
                            o_ps = ps_o.tile([P, LV], F32, tag="o_ps", name="o_ps")
                            for kc in range(KF):
                                nc.tensor.matmul(
                                    o_ps[:],
                                    attn[:, kc, qc * P : (qc + 1) * P],
                                    vt[:, kc],
                                    start=(kc == 0),
                                    stop=(kc == KF - 1),
                                )
                        o_sb = work.tile([P, LV], F32, tag="o_sb")
                        if qc % 2 == 0:
                            nc.scalar.copy(o_sb[:], o_ps[:])
                        else:
                            nc.vector.tensor_copy(o_sb[:], o_ps[:])
                        if not compute_only:
                            nc.sync.dma_start(
                                o_d.ap()[b, qc * P : (qc + 1) * P], o_sb[:]
                            )
    return nc


_CACHED_NC = None


def _get_nc():
    global _CACHED_NC
    if _CACHED_NC is None:
        nc = build_kernel()
        nc.finalize()
        _CACHED_NC = nc
    return _CACHED_NC


def run_sharded(queries, keys, values, W1, trace=False):
    from concourse.bass_utils import run_bass_kernel_spmd

    nc = _get_nc()
    w1t = np.ascontiguousarray(W1.T)  # [512, 128]
    in_maps = []
    for c in range(NCORES):
        sl = slice(c * NB, (c + 1) * NB)
        in_maps.append(
            {
                "queries": np.ascontiguousarray(queries[sl]),
                "keys": np.ascontiguousarray(keys[sl]),
                "values": np.ascontiguousarray(values[sl]),
                "w1t": w1t,
            }
        )
    # the axon tunnel occasionally throws a transient INTERNAL error on the
    # first execution after a device reset; retry a couple of times
    last_exc = None
    for attempt in range(3):
        try:
            bkr = run_bass_kernel_spmd(nc, in_maps, list(range(NCORES)), trace=trace)
            break
        except Exception as e:  # noqa: BLE001
            last_exc = e
            import time

            time.sleep(2.0 * (attempt + 1))
    else:
        raise last_exc
    out = np.concatenate([r["out"] for r in bkr.results], axis=0)
    return out, bkr


def kernel(queries, keys, values, W1):
    queries = np.asarray(queries, dtype=np.float32)
    keys = np.asarray(keys, dtype=np.float32)
    values = np.asarray(values, dtype=np.float32)
    W1 = np.asarray(W1, dtype=np.float32)
    out, _ = run_sharded(queries, keys, values, W1)
    return out



# revision 21
# speedup vs baseline: 1.3354x; 1.3354x over previous
"""Trainium2 Bass kernel for nn_DotProductAttention_57853209477398.

Reference computation (per batch b):
    kprojT = W1 @ keys[b].T                      # [128, 1024]   (d, f)
    scoresT[k, q] = sum_d kprojT[d,k] * Q[d,q]   # [1024, 1024]  (Fk, Fq) / sqrt(128)
    attnT = softmax over q (free dim)            # softmax axis=1 of scores == free dim of scoresT
    out[q, l] = sum_k attnT[k,q] * V[l,k]        # [1024, 512]

Sharding: data-parallel over batch B=32 across 8 cores (4 batches/core).
All matmuls run as float32r (~13-bit-mantissa rounded fp32) at full PE rate.
"""

import numpy as np

import concourse.bacc as bacc
import concourse.mybir as mybir
from concourse.tile import TileContext
from concourse.masks import make_identity

B, LQ, FQ = 32, 128, 1024
LK, FK = 512, 1024
LV, FV = 512, 1024
DP = 128

NCORES = 8
NB = B // NCORES  # batches per core
P = 128
N = 512  # max fp32 moving free dim per matmul
F32 = mybir.dt.float32
F32R = mybir.dt.float32r
INV_SQRT_D = 1.0 / np.sqrt(np.float32(DP))

KC = LK // P  # 4: l-chunks of keys/values rows
KF = FK // P  # 8: k-chunks (scoresT partition tiles)
QC = FQ // P  # 8: q-chunks of the output


def build_kernel(reps=1, dma_only=False, compute_only=False):
    nc = bacc.Bacc("TRN2", target_bir_lowering=False)
    q_d = nc.dram_tensor("queries", [NB, LQ, FQ], F32, kind="ExternalInput")
    k_d = nc.dram_tensor("keys", [NB, LK, FK], F32, kind="ExternalInput")
    v_d = nc.dram_tensor("values", [NB, LV, FV], F32, kind="ExternalInput")
    w_d = nc.dram_tensor("w1t", [LK, DP], F32, kind="ExternalInput")
    o_d = nc.dram_tensor("out", [NB, FQ, LV], F32, kind="ExternalOutput")

    with TileContext(nc) as tc:
        with (
            tc.tile_pool(name="const", bufs=1) as cpool,
            tc.tile_pool(name="io", bufs=2) as io,
            tc.tile_pool(name="work", bufs=2) as work,
            tc.tile_pool(name="ps_big", bufs=2, space="PSUM") as ps_big,
            tc.tile_pool(name="ps_vt", bufs=2, space="PSUM") as ps_vt,
            tc.tile_pool(name="ps_o", bufs=2, space="PSUM") as ps_o,
        ):
            ident = cpool.tile([P, P], F32)
            make_identity(nc, ident[:])
            w1t_raw = cpool.tile([P, KC, DP], F32)
            nc.sync.dma_start(w1t_raw[:], w_d.ap().rearrange("(c p) d -> p c d", p=P))
            w1t = cpool.tile([P, KC, DP], F32R)
            nc.vector.tensor_copy(w1t[:], w1t_raw[:])  # round to f32r
            o_static = cpool.tile([P, LV], F32)
            nc.gpsimd.memset(o_static[:], 0.5)

            import contextlib

            loop_cm = (
                tc.For_i(0, reps, 1, name="rep")
                if reps > 1
                else contextlib.nullcontext()
            )
            with loop_cm:
                for b in range(NB):
                    # fp32 DMA lands in staging tiles; a DVE copy rounds into
                    # separate f32r tiles (FP32r matmul inputs must be produced
                    # by a rounding compute op, not DMA). K chunks are loaded
                    # first: kprojT is the first consumer.
                    kr = work.tile([P, KC, FK], F32R, tag="kr")
                    k_src = k_d.ap()[b].rearrange("(c p) f -> p c f", p=P)
                    for lc in range(KC):
                        for h in range(2):
                            sth = io.tile([P, N], F32, tag="stageh", bufs=4, name="sth")
                            if compute_only:
                                nc.gpsimd.memset(sth[:], 0.25)
                            else:
                                nc.sync.dma_start(sth[:], k_src[:, lc, h * N : (h + 1) * N])
                            if not dma_only:
                                nc.vector.tensor_copy(
                                    kr[:, lc, h * N : (h + 1) * N], sth[:]
                                )

                    qr = work.tile([P, FQ], F32R, tag="qr", bufs=1)
                    st = io.tile([P, FQ], F32, tag="stage", bufs=2)
                    if compute_only:
                        nc.gpsimd.memset(st[:], 0.25)
                    else:
                        nc.sync.dma_start(st[:], q_d.ap()[b])
                    if not dma_only:
                        nc.vector.tensor_copy(qr[:], st[:])

                    v = io.tile([P, KC, FV], F32, tag="v")
                    v_src = v_d.ap()[b].rearrange("(c p) f -> p c f", p=P)
                    for lc in range(KC):
                        if compute_only:
                            nc.gpsimd.memset(v[:, lc], 0.25)
                        else:
                            nc.sync.dma_start(v[:, lc], v_src[:, lc])

                    if dma_only:
                        for qc in range(QC):
                            nc.sync.dma_start(
                                o_d.ap()[b, qc * P : (qc + 1) * P], o_static[:]
                            )
                        continue
                    # ---- kprojT[d, f] = sum_l W1T[l, d] * K[l, f] ----
                    kp_ps = ps_big.tile([P, FK], F32, tag="big")
                    for lc in range(KC):  # 4 (chunk-major: start on first K chunk)
                        for fc in range(FK // N):  # 2
                            nc.tensor.matmul(
                                kp_ps[:, fc * N : (fc + 1) * N],
                                w1t[:, lc],
                                kr[:, lc, fc * N : (fc + 1) * N],
                                start=(lc == 0),
                                stop=(lc == KC - 1),
                            )
                    kproj = work.tile([P, FK], F32R, tag="kproj", bufs=1)
                    # PSUM->SBUF + round to f32r; split across DVE/ACT to
                    # halve the critical-path latency into the scores matmuls
                    nc.vector.tensor_copy(kproj[:, :N], kp_ps[:, :N])
                    nc.scalar.copy(kproj[:, N:], kp_ps[:, N:])

                    # ---- scoresT -> exp/row-sum -> per-kc recip -> scaled V^T ----
                    attn = work.tile([P, KF, FQ], F32R, tag="attn")
                    den = work.tile([P, KF], F32, tag="den")
                    rden = work.tile([P, KF], F32, tag="rden")
                    vt = work.tile([P, KF, LV], F32R, tag="vt")
                    o_half = [
                        ps_o.tile([P, LV], F32, tag="o_ps", name=f"o_half{qg}")
                        for qg in range(2)
                    ]
                    for kc in range(KF):  # 8
                        sc_ps = ps_big.tile([P, FQ], F32, tag="big")
                        for qc in range(FQ // N):  # 2
                            nc.tensor.matmul(
                                sc_ps[:, qc * N : (qc + 1) * N],
                                kproj[:, kc * P : (kc + 1) * P],
                                qr[:, qc * N : (qc + 1) * N],
                                start=True,
                                stop=True,
                            )
                        nc.scalar.activation(
                            attn[:, kc],
                            sc_ps[:],
                            mybir.ActivationFunctionType.Exp,
                            scale=float(INV_SQRT_D),
                            accum_out=den[:, kc : kc + 1],
                        )
                        nc.vector.reciprocal(rden[:, kc : kc + 1], den[:, kc : kc + 1])
                        vt_ps = ps_vt.tile([P, LV], F32, tag="vt_ps")
                        for lc in range(KC):
                            nc.tensor.transpose(
                                vt_ps[:, lc * P : (lc + 1) * P],
                                v[:, lc, kc * P : (kc + 1) * P],
                                ident[:],
                            )
                        nc.vector.tensor_scalar_mul(
                            vt[:, kc], vt_ps[:], rden[:, kc : kc + 1]
                        )
                        # first-half out matmuls ride along per kc: fills PE
                        # during the ACT-paced exp chain and shortens the
                        # batch tail to the remaining half
                        for qg in range(2):
                            nc.tensor.matmul(
                                o_half[qg],
                                attn[:, kc, qg * P : (qg + 1) * P],
                                vt[:, kc],
                                start=(kc == 0),
                                stop=(kc == KF - 1),
                            )

                    # ---- out[q, l] = sum_k attnT[k, q] * VT[k, l] ----
                    for qc in range(QC):  # 8
                        if qc < 2:
                            o_ps = o_half[qc]
                        else:
                            o_ps = ps_o.tile([P, LV], F32, tag="o_ps", name="o_ps")
                            for kc in range(KF):
                                nc.tensor.matmul(
                                    o_ps[:],
                                    attn[:, kc, qc * P : (qc + 1) * P],
                                    vt[:, kc],
                                    start=(kc == 0),
                                    stop=(kc == KF - 1),
                                )
                        o_sb = work.tile([P, LV], F32, tag="o_sb")
                        if qc % 2 == 0:
                            nc.scalar.copy(o_sb[:], o_ps[:])
                        else:
                            nc.vector.tensor_copy(o_sb[:], o_ps[:])
                        if not compute_only:
                            nc.sync.dma_start(
                                o_d.ap()[b, qc * P : (qc + 1) * P], o_sb[:]
                            )
    return nc


_CACHED_NC = None


def _get_nc():
    global _CACHED_NC
    if _CACHED_NC is None:
        nc = build_kernel()
        nc.finalize()
        _CACHED_NC = nc
    return _CACHED_NC


def run_sharded(queries, keys, values, W1, trace=False):
    from concourse.bass_utils import run_bass_kernel_spmd

    nc = _get_nc()
    w1t = np.ascontiguousarray(W1.T)  # [512, 128]
    in_maps = []
    for c in range(NCORES):
        sl = slice(c * NB, (c + 1) * NB)
        in_maps.append(
            {
                "queries": np.ascontiguousarray(queries[sl]),
                "keys": np.ascontiguousarray(keys[sl]),
                "values": np.ascontiguousarray(values[sl]),
                "w1t": w1t,
            }
        )
    # the axon tunnel occasionally throws a transient INTERNAL error on the
    # first execution after a device reset; retry a couple of times
    last_exc = None
    for attempt in range(3):
        try:
            bkr = run_bass_kernel_spmd(nc, in_maps, list(range(NCORES)), trace=trace)
            break
        except Exception as e:  # noqa: BLE001
            last_exc = e
            import time

            time.sleep(2.0 * (attempt + 1))
    else:
        raise last_exc
    out = np.concatenate([r["out"] for r in bkr.results], axis=0)
    return out, bkr


def kernel(queries, keys, values, W1):
    queries = np.asarray(queries, dtype=np.float32)
    keys = np.asarray(keys, dtype=np.float32)
    values = np.asarray(values, dtype=np.float32)
    W1 = np.asarray(W1, dtype=np.float32)
    out, _ = run_sharded(queries, keys, values, W1)
    return out

